# revision 16
# baseline (speedup 1.0000x reference)
"""GCNII backbone on 8 Trainium2 NeuronCores (Bass/Tile SPMD).

Strategy (graph/data parallel, per sharding hint):
  - Nodes are partitioned contiguously across the 8 cores; each core owns
    N/8 destination rows of every per-node tensor.
  - The layer state h is held as a replicated fp16 "gather table"
    g = dis (.) h in DRAM (dis = deg^-1/2), rebuilt each layer by AllGather
    of the per-core slices.
  - segment_sum aggregation: edges are grouped by destination block
    (128 dests); per 128-edge chunk, the source rows are fetched with the
    MoE dma_gather instruction and scatter-reduced into PSUM with a one-hot
    matmul  psum[f, d] += M[e, f]^T @ S[e, d],  where S[e, d] =
    0.5 * dis[col_e] * (dest_local_e == d) is built on the vector engine.
    The symmetric norm enorm = dis[row]*dis[col] is separable: the row
    factor is folded into the gather table, the col factor into S.
  - dense GCN2Conv algebra:  h' = (1-b)u + b(u W1 + r (W2-W1)), u = t + r,
    done in feature-major layout with PE matmuls; graph-LayerNorm global
    stats via bn_stats + a tiny AllReduce; relu + affine fused on ACT.
  - int16 gather indices cap tables at 32768 rows, so edges are split into
    a "lo" stream (src < 32768) and "hi" stream (src >= 32768) that gather
    from offset views of the table.

kernel(**inputs) takes the full unsharded inputs and returns the full
[N, DOUT] fp32 output; all sharding happens inside.
"""

import sys

sys.path.insert(0, "/opt/trn_rl_repo")

import math
from contextlib import ExitStack
from dataclasses import dataclass

import numpy as np

import concourse.bass as bass
import concourse.tile as tile
from concourse import bacc, mybir
from concourse.bass_utils import run_bass_kernel_spmd
from concourse.masks import make_identity

F32 = mybir.dt.float32
F16 = mybir.dt.float16
I16 = mybir.dt.int16

ALPHA = 0.5
THETA = 1.0
EPS = 1e-5
SENT = 300.0  # dest-local sentinel for padded edge slots (never matches iota)


@dataclass
class Cfg:
    N: int = 50000
    E: int = 800000
    DIN: int = 128
    DH: int = 256
    DOUT: int = 128
    L: int = 8
    NCORES: int = 8
    LO_LIM: int = 32768
    GCH: int = 8  # chunks (of 128 edges) per dma_gather granule
    DBG: bool = False

    @property
    def NPB(self):
        return self.N // self.NCORES

    @property
    def NB(self):
        return (self.NPB + 127) // 128

    def nb(self, b):
        return min(128, self.NPB - b * 128)

    @property
    def slices(self):
        out = []
        st = 0
        while st < self.NPB:
            w = min(512, self.NPB - st)
            out.append((st, w))
            st += w
        return out


def _wrap16(flat):
    """dma_gather index layout: element i -> [i % 16, i // 16], replicated
    to 128 partitions."""
    t = len(flat)
    arr = flat.reshape(t // 16, 16).T.astype(np.int16)  # [16, t/16]
    return np.tile(arr, (8, 1))


def build_meta(cfg: Cfg, edge_index: np.ndarray):
    """Host-side preprocessing: degrees, per-core edge schedules, index and
    one-hot metadata arrays. Returns (sched, per_core_inputs)."""
    N, NPB, NB, GCH = cfg.N, cfg.NPB, cfg.NB, cfg.GCH
    loop = np.arange(N, dtype=np.int64)
    row = np.concatenate([edge_index[0].astype(np.int64), loop])
    col = np.concatenate([edge_index[1].astype(np.int64), loop])
    deg = np.bincount(col, minlength=N).astype(np.float64)
    dis = (deg ** -0.5).astype(np.float32)  # deg >= 1 thanks to self loops

    # per-core, per-(block, stream) edge lists
    order = np.argsort(col, kind="stable")
    row_s, col_s = row[order], col[order]
    w_s = (0.5 * dis[col_s]).astype(np.float32)  # (1-ALPHA) * dis[col]
    core_of = col_s // NPB
    core_start = np.searchsorted(core_of, np.arange(cfg.NCORES + 1))

    percore = []
    c_lo = c_hi = 0
    for k in range(cfg.NCORES):
        lo_blocks, hi_blocks = [], []
        s, e = core_start[k], core_start[k + 1]
        r_k = row_s[s:e]
        c_k = col_s[s:e] - k * NPB
        w_k = w_s[s:e]
        b_k = c_k // 128
        dl_k = c_k % 128
        hi_m = r_k >= cfg.LO_LIM
        for b in range(NB):
            bm = b_k == b
            for m, lst, off in ((bm & ~hi_m, lo_blocks, 0), (bm & hi_m, hi_blocks, cfg.LO_LIM)):
                lst.append((r_k[m] - off, dl_k[m], w_k[m]))
        percore.append((lo_blocks, hi_blocks))
        c_lo = max(c_lo, max(-(-len(x[0]) // 128) for x in lo_blocks))
        c_hi = max(c_hi, max(-(-len(x[0]) // 128) for x in hi_blocks))
    c_lo, c_hi = max(c_lo, 1), max(c_hi, 1)
    g_lo = -(-(NB * c_lo) // GCH)
    g_hi = -(-(NB * c_hi) // GCH)

    # core-independent chunk schedule: per block, its chunks as
    # (stream, granule, slot_in_granule, stream_chunk_index)
    sched_blocks = []
    for b in range(NB):
        ch = []
        for j in range(c_lo):
            q = b * c_lo + j
            ch.append((0, q // GCH, q % GCH, q))
        for j in range(c_hi):
            q = b * c_hi + j
            ch.append((1, q // GCH, q % GCH, q))
        sched_blocks.append(ch)
    sched = dict(C=(c_lo, c_hi), G=(g_lo, g_hi), blocks=sched_blocks)

    per_inputs = []
    for k in range(cfg.NCORES):
        maps = {}
        for s, (blocks, c_s, g_s) in enumerate(
            ((percore[k][0], c_lo, g_lo), (percore[k][1], c_hi, g_hi))
        ):
            tq = g_s * GCH  # total stream chunks incl. trailing pad
            idx = np.zeros(tq * 128, np.int64)
            dca = np.full((128, tq), SENT, np.float32)
            dcw = np.zeros((128, tq), np.float32)
            for b in range(NB):
                rr, dd, ww = blocks[b]
                n = len(rr)
                for j in range(c_s):
                    q = b * c_s + j
                    sl = slice(j * 128, min((j + 1) * 128, n))
                    cnt = max(0, sl.stop - sl.start)
                    if cnt <= 0:
                        continue
                    idx[q * 128 : q * 128 + cnt] = rr[sl]
                    dca[:cnt, q] = dd[sl]
                    dcw[:cnt, q] = ww[sl]
            maps[f"idx{s}"] = _wrap16(idx)
            maps[f"dca{s}"] = dca
            maps[f"dcw{s}"] = dcw.astype(np.float16)
        v = dis[k * NPB : (k + 1) * NPB]
        vp = np.zeros(NB * 128, np.float32)
        vp[: len(v)] = v
        dnm = vp.reshape(NB, 128).T  # dnm[p, b] = dis[base + 128 b + p]
        maps["disn"] = np.ascontiguousarray(dnm)
        maps["dis2n"] = np.ascontiguousarray(2.0 * dnm)
        per_inputs.append(maps)
    return sched, per_inputs


def build_program(cfg: Cfg, sched):
    NB, GCH, DH, NPB, L = cfg.NB, cfg.GCH, cfg.DH, cfg.NPB, cfg.L
    c_lo, c_hi = sched["C"]
    g_lo, g_hi = sched["G"]
    n_gr = (g_lo, g_hi)
    slices = cfg.slices
    betas = [float(np.log(THETA / l + 1.0)) for l in range(1, L + 1)]

    nc = bacc.Bacc("TRN2", target_bir_lowering=False, debug=False, num_devices=cfg.NCORES)

    # ---- I/O ----
    xT = nc.declare_dram_parameter("xT", [cfg.DIN, NPB], F32, isOutput=False)
    idxp = [
        nc.declare_dram_parameter(f"idx{s}", [128, n_gr[s] * GCH * 8], I16, isOutput=False)
        for s in range(2)
    ]
    dcap = [
        nc.declare_dram_parameter(f"dca{s}", [128, n_gr[s] * GCH], F32, isOutput=False)
        for s in range(2)
    ]
    dcwp = [
        nc.declare_dram_parameter(f"dcw{s}", [128, n_gr[s] * GCH], F16, isOutput=False)
        for s in range(2)
    ]
    disn = nc.declare_dram_parameter("disn", [128, NB], F32, isOutput=False)
    dis2n = nc.declare_dram_parameter("dis2n", [128, NB], F32, isOutput=False)
    lin1p = nc.declare_dram_parameter("lin1", [cfg.DIN, DH], F32, isOutput=False)
    b1p = nc.declare_dram_parameter("b1h", [128, 2], F32, isOutput=False)
    w1p = nc.declare_dram_parameter("w1", [128, L, 2, DH], F32, isOutput=False)
    wdp = nc.declare_dram_parameter("wd", [128, L, 2, DH], F32, isOutput=False)
    gamp = nc.declare_dram_parameter("gam", [128, L, 2], F32, isOutput=False)
    betp = nc.declare_dram_parameter("bet", [128, L, 2], F32, isOutput=False)
    lin2p = nc.declare_dram_parameter("lin2", [128, 2, cfg.DOUT], F32, isOutput=False)
    b2p = nc.declare_dram_parameter("b2h", [128, 1], F32, isOutput=False)
    iotp = nc.declare_dram_parameter("iota", [128, 128], F32, isOutput=False)
    outp = nc.declare_dram_parameter("out", [128, NPB], F32, isOutput=True)
    dbgp = {}
    if cfg.DBG:
        for nm in ("rT", "u0", "h0", "hp", "mm"):
            dbgp[nm] = nc.declare_dram_parameter(nm, [128, 2, NPB], F32, isOutput=True)
        dbgp["sbc"] = nc.declare_dram_parameter("sbc", [128, 8], F32, isOutput=True)

    rep = [list(range(cfg.NCORES))]

    with tile.TileContext(nc) as tc, ExitStack() as ctx:
        cons = ctx.enter_context(tc.tile_pool(name="cons", bufs=1))
        big = ctx.enter_context(tc.tile_pool(name="big", bufs=1))
        mpools = [
            ctx.enter_context(tc.tile_pool(name=f"m{s}", bufs=b_)) for s, b_ in ((0, 3), (1, 2))
        ]
        spools = [
            ctx.enter_context(tc.tile_pool(name=f"s{s}", bufs=2)) for s in range(2)
        ]
        work = ctx.enter_context(tc.tile_pool(name="work", bufs=3))
        wl = ctx.enter_context(tc.tile_pool(name="wl", bufs=2))
        pa = ctx.enter_context(tc.tile_pool(name="pa", bufs=4, space="PSUM"))
        pd = ctx.enter_context(tc.tile_pool(name="pd", bufs=3, space="PSUM"))
        dram = ctx.enter_context(tc.tile_pool(name="dram", bufs=1, space="DRAM"))

        # ---- resident SBUF state ----
        uhT = big.tile([128, 2, NPB], F32)  # u^T, later h^T (feature-major)
        rT = big.tile([128, 2, NPB], F32)  # r^T = 0.5 x0^T

        idx_sb = [cons.tile([128, n_gr[s] * GCH * 8], I16, name=f"idxs{s}") for s in range(2)]
        dca_sb = [cons.tile([128, n_gr[s] * GCH], F32, name=f"dcas{s}") for s in range(2)]
        dcw_sb = [cons.tile([128, n_gr[s] * GCH], F16, name=f"dcws{s}") for s in range(2)]
        for s in range(2):
            nc.sync.dma_start(out=idx_sb[s][:], in_=idxp[s][:])
            nc.sync.dma_start(out=dca_sb[s][:], in_=dcap[s][:])
            nc.sync.dma_start(out=dcw_sb[s][:], in_=dcwp[s][:])
        disn_sb = cons.tile([128, NB], F32)
        nc.sync.dma_start(out=disn_sb[:], in_=disn[:])
        dis2n_sb = cons.tile([128, NB], F32)
        nc.sync.dma_start(out=dis2n_sb[:], in_=dis2n[:])
        lin1_sb = cons.tile([cfg.DIN, DH], F32)
        nc.sync.dma_start(out=lin1_sb[:], in_=lin1p[:])
        b1_sb = cons.tile([128, 2], F32)
        nc.sync.dma_start(out=b1_sb[:], in_=b1p[:])
        gam_sb = cons.tile([128, L, 2], F32)
        nc.sync.dma_start(out=gam_sb[:], in_=gamp[:])
        bet_sb = cons.tile([128, L, 2], F32)
        nc.sync.dma_start(out=bet_sb[:], in_=betp[:])
        lin2_sb = cons.tile([128, 2, cfg.DOUT], F32)
        nc.sync.dma_start(out=lin2_sb[:], in_=lin2p[:])
        b2_sb = cons.tile([128, 1], F32)
        nc.sync.dma_start(out=b2_sb[:], in_=b2p[:])
        iota_sb = cons.tile([128, 128], F32)
        nc.sync.dma_start(out=iota_sb[:], in_=iotp[:])
        ident = cons.tile([128, 128], F32)
        make_identity(nc, ident[:])
        ones_sb = cons.tile([128, 1], F32)
        nc.vector.memset(ones_sb[:], 1.0)

        # ---- DRAM internals ----
        g_tab = [
            dram.tile([cfg.N, DH], F16, addr_space="Shared", name=f"gt{i}")
            for i in range(L)
        ]
        g_slc = [dram.tile([NPB, DH], F16, name=f"gs{i}") for i in range(L)]
        ln_in = [dram.tile([1, 8], F32, name=f"li{i}") for i in range(L)]
        ln_out = [
            dram.tile([1, 8], F32, addr_space="Shared", name=f"lo{i}") for i in range(L)
        ]

        def bc(ap_base, extra, where):
            """append/inject a broadcast dim into a 2D AP"""
            a = ap_base.ap
            if where == "tail":
                pat = [a[0], a[1], [0, extra]]
            else:  # mid
                pat = [a[0], [0, extra], a[1]]
            return bass.AP(tensor=ap_base.tensor, offset=ap_base.offset, ap=pat)

        def g_emit(src, scale_nm, li):
            """transpose src[ft][:, block] -> node-major fp16 g slice, AllGather."""
            for b in range(NB):
                nb_ = cfg.nb(b)
                gnm = work.tile([128, DH], F16, tag="gnm", name=f"gnm{li}_{b}")
                for ft in range(2):
                    pt = pa.tile([128, 128], F32, tag="pa", name=f"pt{li}_{b}_{ft}")
                    nc.tensor.transpose(
                        pt[:nb_, :], src[:, ft, b * 128 : b * 128 + nb_], ident[:]
                    )
                    nc.scalar.activation(
                        gnm[:nb_, ft * 128 : (ft + 1) * 128],
                        pt[:nb_, :],
                        mybir.ActivationFunctionType.Copy,
                        scale=scale_nm[:nb_, b : b + 1],
                    )
                nc.sync.dma_start(
                    out=g_slc[li][b * 128 : b * 128 + nb_, :], in_=gnm[:nb_, :]
                )
            nc.gpsimd.collective_compute(
                "AllGather",
                mybir.AluOpType.bypass,
                replica_groups=rep,
                ins=[g_slc[li][:]],
                outs=[g_tab[li][:]],
            )

        # ---- pre-pass: r^T = 0.5 relu(x @ lin1 + b1) ----
        for st, w in slices:
            xsl = work.tile([128, 512], F32, tag="xsl", name=f"xsl{st}")
            nc.sync.dma_start(out=xsl[:, :w], in_=xT[:, st : st + w])
            for fo in range(2):
                pm = pd.tile([128, 512], F32, tag="pd", name=f"pp{st}_{fo}")
                nc.tensor.matmul(
                    pm[:, :w],
                    lin1_sb[:, fo * 128 : (fo + 1) * 128],
                    xsl[:, :w],
                    start=True,
                    stop=True,
                )
                nc.scalar.activation(
                    rT[:, fo, st : st + w],
                    pm[:, :w],
                    mybir.ActivationFunctionType.Relu,
                    bias=b1_sb[:, fo : fo + 1],
                    scale=0.5,
                )
        g_emit(rT, dis2n_sb, 0)
        if cfg.DBG:
            nc.sync.dma_start(out=dbgp["rT"][:], in_=rT[:])

        # ---- layers ----
        for li in range(L):
            bl = betas[li]
            w1_sb = wl.tile([128, 2, DH], F32, tag="w1", name=f"w1_{li}")
            nc.sync.dma_start(out=w1_sb[:], in_=w1p[:, li, :, :])
            wd_sb = wl.tile([128, 2, DH], F32, tag="wd", name=f"wd_{li}")
            nc.sync.dma_start(out=wd_sb[:], in_=wdp[:, li, :, :])

            # -- aggregation --
            mt = [dict(), dict()]
            st_ = [dict(), dict()]

            def granule(s, g, li=li, mt=mt, st_=st_):
                if g in mt[s]:
                    return
                m = mpools[s].tile([128, GCH, DH], F16, tag=f"m{s}", name=f"m{s}_{li}_{g}")
                src = (
                    g_tab[li][0 : cfg.LO_LIM, :]
                    if s == 0
                    else g_tab[li][cfg.LO_LIM : cfg.N, :]
                )
                nc.gpsimd.dma_gather(
                    m[:], src, idx_sb[s][:, g * GCH * 8 : (g + 1) * GCH * 8],
                    GCH * 128, GCH * 128, DH,
                )
                sg = spools[s].tile([128, GCH, 128], F16, tag=f"s{s}", name=f"sg{s}_{li}_{g}")
                dsl = dca_sb[s][:, g * GCH : (g + 1) * GCH]
                wsl = dcw_sb[s][:, g * GCH : (g + 1) * GCH]
                nc.vector.tensor_tensor(
                    out=sg[:],
                    in0=bc(dsl, 128, "tail"),
                    in1=bc(iota_sb[:], GCH, "mid"),
                    op=mybir.AluOpType.is_equal,
                )
                nc.vector.tensor_tensor(
                    out=sg[:], in0=sg[:], in1=bc(wsl, 128, "tail"), op=mybir.AluOpType.mult
                )
                mt[s][g] = m
                st_[s][g] = sg

            for b in range(NB):
                chunks = sched["blocks"][b]
                pg = [
                    pa.tile([128, 128], F32, tag="pa", name=f"pg{li}_{b}_{fb}")
                    for fb in range(2)
                ]
                last = len(chunks) - 1
                for pos, (s, g, sg_i, _q) in enumerate(chunks):
                    granule(s, g)
                    for fb in range(2):
                        nc.tensor.matmul(
                            pg[fb][:],
                            mt[s][g][:, sg_i, fb * 128 : (fb + 1) * 128],
                            st_[s][g][:, sg_i, :],
                            start=(pos == 0),
                            stop=(pos == last),
                        )
                nb_ = cfg.nb(b)
                for ft in range(2):
                    nc.vector.tensor_add(
                        uhT[:, ft, b * 128 : b * 128 + nb_],
                        pg[ft][:, :nb_],
                        rT[:, ft, b * 128 : b * 128 + nb_],
                    )

            if cfg.DBG and li == 0:
                nc.sync.dma_start(out=dbgp["u0"][:], in_=uhT[:])

            # -- dense: h = (1-b) u + b (u W1 + r Wd) --
            for st, w in slices:
                pms = []
                for fo in range(2):
                    pm = pd.tile([128, 512], F32, tag="pd", name=f"pm{li}_{st}_{fo}")
                    for wi, (wsb, srct) in enumerate(((w1_sb, uhT), (wd_sb, rT))):
                        for kb in range(2):
                            nc.tensor.matmul(
                                pm[:, :w],
                                wsb[:, kb, fo * 128 : (fo + 1) * 128],
                                srct[:, kb, st : st + w],
                                start=(wi == 0 and kb == 0),
                                stop=(wi == 1 and kb == 1),
                            )
                    pms.append(pm)
                for fo in range(2):
                    msb = work.tile([128, 512], F32, tag="msb", name=f"ms{li}_{st}_{fo}")
                    nc.scalar.mul(msb[:, :w], pms[fo][:, :w], bl)
                    if cfg.DBG and li == 0:
                        nc.sync.dma_start(out=dbgp["mm"][:, fo, st : st + w], in_=msb[:, :w])
                    nc.vector.tensor_scalar_mul(
                        uhT[:, fo, st : st + w], uhT[:, fo, st : st + w], 1.0 - bl
                    )
                    nc.vector.tensor_add(
                        uhT[:, fo, st : st + w], uhT[:, fo, st : st + w], msb[:, :w]
                    )

            if cfg.DBG and li == 0:
                nc.sync.dma_start(out=dbgp["hp"][:], in_=uhT[:])

            # -- graph layernorm stats --
            ps_s = pa.tile([1, 2], F32, tag="pa", name=f"pst{li}")
            for ft in range(2):
                stt = work.tile([128, len(slices), 6], F32, tag="bnst", name=f"bn{li}_{ft}")
                for i, (st, w) in enumerate(slices):
                    nc.vector.bn_stats(out=stt[:, i, :], in_=uhT[:, ft, st : st + w])
                mv = work.tile([128, 2], F32, tag="mv", name=f"mv{li}_{ft}")
                nc.vector.bn_aggr(out=mv[:], in_=stt[:])
                stk = work.tile([128, 2], F32, tag="stk", name=f"stk{li}_{ft}")
                nc.vector.tensor_copy(stk[:, 0:1], mv[:, 0:1])
                nc.vector.tensor_tensor(
                    out=stk[:, 1:2], in0=mv[:, 0:1], in1=mv[:, 0:1], op=mybir.AluOpType.mult
                )
                nc.vector.tensor_add(stk[:, 1:2], stk[:, 1:2], mv[:, 1:2])
                nc.tensor.matmul(
                    ps_s[:], ones_sb[:], stk[:], start=(ft == 0), stop=(ft == 1)
                )
            lnsb = work.tile([1, 8], F32, tag="lnsb", name=f"ln{li}")
            nc.vector.memset(lnsb[:], 0.0)
            nc.vector.tensor_copy(lnsb[:, 0:2], ps_s[:])
            nc.sync.dma_start(out=ln_in[li][:], in_=lnsb[:])
            nc.gpsimd.collective_compute(
                "AllReduce",
                mybir.AluOpType.add,
                replica_groups=rep,
                ins=[ln_in[li][:]],
                outs=[ln_out[li][:]],
            )
            stbc = work.tile([128, 8], F32, tag="stbc", name=f"sb{li}")
            lo_ap = ln_out[li][:]
            nc.gpsimd.dma_start(
                out=stbc[:],
                in_=bass.AP(tensor=lo_ap.tensor, offset=lo_ap.offset, ap=[[0, 128], [1, 8]]),
            )
            if cfg.DBG and li == 0:
                nc.sync.dma_start(out=dbgp["sbc"][:], in_=stbc[:])
            inv_n = 1.0 / (2.0 * 128.0 * cfg.NCORES)
            mu = work.tile([128, 1], F32, tag="mu", name=f"mu{li}")
            nc.vector.tensor_scalar_mul(mu[:], stbc[:, 0:1], inv_n)
            tmp = work.tile([128, 1], F32, tag="tmp", name=f"tq{li}")
            nc.vector.tensor_scalar_mul(tmp[:], stbc[:, 1:2], inv_n)  # E[h^2]
            sq = work.tile([128, 1], F32, tag="sq2", name=f"sq{li}")
            nc.vector.tensor_tensor(out=sq[:], in0=mu[:], in1=mu[:], op=mybir.AluOpType.mult)
            nc.vector.tensor_tensor(
                out=tmp[:], in0=tmp[:], in1=sq[:], op=mybir.AluOpType.subtract
            )
            nc.scalar.sqrt(tmp[:], tmp[:])
            nc.vector.tensor_scalar_add(tmp[:], tmp[:], EPS)
            inv = work.tile([128, 1], F32, tag="inv", name=f"iv{li}")
            nc.vector.reciprocal(inv[:], tmp[:])
            for ft in range(2):
                scl = work.tile([128, 1], F32, tag="scl", name=f"sc{li}_{ft}")
                nc.vector.tensor_tensor(
                    out=scl[:], in0=gam_sb[:, li, ft : ft + 1], in1=inv[:],
                    op=mybir.AluOpType.mult,
                )
                bia = work.tile([128, 1], F32, tag="bia", name=f"bi{li}_{ft}")
                nc.vector.tensor_tensor(
                    out=bia[:], in0=scl[:], in1=mu[:], op=mybir.AluOpType.mult
                )
                nc.vector.tensor_tensor(
                    out=bia[:], in0=bet_sb[:, li, ft : ft + 1], in1=bia[:],
                    op=mybir.AluOpType.subtract,
                )
                nc.scalar.activation(
                    uhT[:, ft, 0:NPB],
                    uhT[:, ft, 0:NPB],
                    mybir.ActivationFunctionType.Relu,
                    bias=bia[:],
                    scale=scl[:],
                )

            if cfg.DBG and li == 0:
                nc.sync.dma_start(out=dbgp["h0"][:], in_=uhT[:])

            if li < L - 1:
                g_emit(uhT, disn_sb, li + 1)

        # ---- final: out^T = relu(lin2^T h^T + b2) ----
        for st, w in slices:
            pm = pd.tile([128, 512], F32, tag="pd", name=f"po{st}")
            for kb in range(2):
                nc.tensor.matmul(
                    pm[:, :w],
                    lin2_sb[:, kb, :],
                    uhT[:, kb, st : st + w],
                    start=(kb == 0),
                    stop=(kb == 1),
                )
            osb = work.tile([128, 512], F32, tag="osb", name=f"ot{st}")
            nc.scalar.activation(
                osb[:, :w],
                pm[:, :w],
                mybir.ActivationFunctionType.Relu,
                bias=b2_sb[:],
                scale=1.0,
            )
            nc.sync.dma_start(out=outp[:, st : st + w], in_=osb[:, :w])

    nc.compile()
    return nc


def make_shared_inputs(cfg: Cfg, lin1_w, lin1_b, conv_w1, conv_w2, norm_gamma, norm_beta, lin2_w, lin2_b):
    L, DH = cfg.L, cfg.DH
    wd = conv_w2 - conv_w1
    w1h = np.zeros((128, L, 2, DH), np.float32)
    wdh = np.zeros((128, L, 2, DH), np.float32)
    for l in range(L):
        for kb in range(2):
            w1h[:, l, kb, :] = conv_w1[l, kb * 128 : (kb + 1) * 128, :]
            wdh[:, l, kb, :] = wd[l, kb * 128 : (kb + 1) * 128, :]
    gamh = np.zeros((128, L, 2), np.float32)
    beth = np.zeros((128, L, 2), np.float32)
    for l in range(L):
        for ft in range(2):
            gamh[:, l, ft] = norm_gamma[l, ft * 128 : (ft + 1) * 128]
            beth[:, l, ft] = norm_beta[l, ft * 128 : (ft + 1) * 128]
    lin2h = np.zeros((128, 2, cfg.DOUT), np.float32)
    for kb in range(2):
        lin2h[:, kb, :] = lin2_w[kb * 128 : (kb + 1) * 128, :]
    b1h = np.stack([0.5 * lin1_b[:128], 0.5 * lin1_b[128:]], 1).astype(np.float32)
    return {
        "lin1": lin1_w.astype(np.float32),
        "b1h": np.ascontiguousarray(b1h),
        "w1": w1h,
        "wd": wdh,
        "gam": gamh,
        "bet": beth,
        "lin2": lin2h,
        "b2h": lin2_b.reshape(128, 1).astype(np.float32),
        "iota": np.tile(np.arange(128, dtype=np.float32), (128, 1)),
    }


_cache = {}


def _get_program(cfg: Cfg, edge_index):
    import hashlib

    key = hashlib.sha256(edge_index.tobytes()).hexdigest() + str(cfg)
    if key not in _cache:
        sched, per_inputs = build_meta(cfg, edge_index)
        nc = build_program(cfg, sched)
        _cache[key] = (nc, per_inputs)
    return _cache[key]


def _kernel_impl(cfg, x, edge_index, lin1_w, lin1_b, conv_w1, conv_w2, norm_gamma, norm_beta, lin2_w, lin2_b):
    x = np.asarray(x, np.float32)
    nc, per_inputs = _get_program(cfg, np.asarray(edge_index, np.int32))
    shared = make_shared_inputs(
        cfg,
        np.asarray(lin1_w, np.float32), np.asarray(lin1_b, np.float32),
        np.asarray(conv_w1, np.float32), np.asarray(conv_w2, np.float32),
        np.asarray(norm_gamma, np.float32), np.asarray(norm_beta, np.float32),
        np.asarray(lin2_w, np.float32), np.asarray(lin2_b, np.float32),
    )
    in_maps = []
    for k in range(cfg.NCORES):
        m = dict(shared)
        m.update(per_inputs[k])
        m["xT"] = np.ascontiguousarray(x[k * cfg.NPB : (k + 1) * cfg.NPB].T)
        in_maps.append(m)
    res = run_bass_kernel_spmd(nc, in_maps, list(range(cfg.NCORES))).results
    return np.concatenate([res[k]["out"].T for k in range(cfg.NCORES)], 0)


def kernel(x, edge_index, lin1_w, lin1_b, conv_w1, conv_w2, norm_gamma, norm_beta, lin2_w, lin2_b):
    return _kernel_impl(
        Cfg(), x, edge_index, lin1_w, lin1_b, conv_w1, conv_w2,
        norm_gamma, norm_beta, lin2_w, lin2_b,
    )
